# revision 74
# baseline (speedup 1.0000x reference)
"""Trainium2 Bass kernel for a vanilla transformer block (nn_BlockVanilla).

  xn  = LN(x; g1, b1)
  q,k,v = xn@Wq+bq, xn@Wk+bk, xn@Wv+bv            (H heads x E)
  h   = softmax(q k^T / sqrt(E)) v                 (per batch, per head)
  y1  = x + h@Wo + bo
  out = y1 + gelu(LN(y1; g2, b2)@W1 + bf1)@W2 + bf2

Sharding: pure data-parallel over rows.  The flattened input is [B*S, D];
core c owns rows [c*R, (c+1)*R).  Attention couples all rows of a batch, so
each core also receives its whole batch's rows ("x_batch") and computes K/V
for all of them locally (replicated-KV) — no collectives.  x_batch is
reordered host-side so the core's OWN rows come first (attention is
permutation-invariant over keys), letting the Q projection reuse the batch
LN output — own rows are never normalized twice.

All matmuls run in bf16 with fp32 PSUM accumulation; LN stays in fp32.

v4 attention:
 - scores run as K=64 matmuls, two heads back-to-back on disjoint PE
   row-halves (tile_position row strips 0/64): the pair executes
   concurrently in the array (~216ns for both at N=512).
 - q-groups are the OUTER loop: per (head-pair, q-group) only two PSUM
   accumulator banks are held, so with bufs=4 the accumulators
   double-buffer across boundaries and eviction never stalls the PE.
 - exp splits by head: even heads get exact ScalarE Exp, odd heads get a
   Schraudolph bit-trick exp on the DVE (int16 bitcast into bf16, ~2%
   err).  Softmax rows stay within one engine so num/den errors cancel.
 - softmax normalization is deferred: a ones-column denominator rides in
   the AV matmul (M=65), h is evicted UN-normalized; denominators go
   through a DRAM round-trip onto partitions 0..H-1, one reciprocal +
   select-matmul broadcast + one DVE multiply finish the job.
 - bk is dropped (softmax is invariant to per-query logit shifts); bv is
   folded through the attention average into bo host-side.
"""

import numpy as np

import concourse.bass as bass
import concourse.mybir as mybir
import concourse.tile as tile
from concourse import bacc
from concourse.bass_utils import run_bass_kernel_spmd
from concourse.masks import make_identity

F32 = mybir.dt.float32
BF16 = mybir.dt.bfloat16
FP8 = mybir.dt.float8e4
I16 = mybir.dt.int16
OP = mybir.AluOpType
ACT = mybir.ActivationFunctionType
DR = mybir.MatmulPerfMode.DoubleRow

# QKV weights are pre-scaled by 2^6 host-side so their ~0.02-magnitude
# entries land in fp8e4m3's normal range; evictions divide it back out.
WSCALE = 64.0

P = 128
EPS = 1e-6

# Schraudolph fast-exp into bf16 bits: exp(x) ~= bf16_bits(int16(x*FE_A+FE_B))
FE_A = 128.0 / float(np.log(2.0))
FE_B = 16250.7


def _ngroups(total, g=512):
    return [(n0, min(g, total - n0)) for n0 in range(0, total, g)]


def build_nc(R=1024, RB=2048, D=1024, H=16, E=64, FF=4096, n_cores=8,
             sim_safe_gelu=False, dve_exp=True, debug_taps=False):
    """Build the per-core Bacc graph.  R: own rows, RB: batch rows."""
    FT = D // P           # feature tiles of D
    RBT = RB // P         # batch row tiles (= attention k tiles)
    FFT = FF // P         # feature tiles of FF
    HPT = P // E          # heads per feature tile
    assert H * E == D and D % P == 0 and R % P == 0 and RB % P == 0

    nc = bacc.Bacc("TRN2", target_bir_lowering=False, debug=False,
                   num_devices=n_cores)

    x_own = nc.dram_tensor("x_own", [R, D], F32, kind="ExternalInput")
    x_batch = nc.dram_tensor("x_batch", [RB, D], F32, kind="ExternalInput")
    Wq = nc.dram_tensor("Wq", [D, D], FP8, kind="ExternalInput")
    Wk = nc.dram_tensor("Wk", [D, D], FP8, kind="ExternalInput")
    Wv = nc.dram_tensor("Wv", [D, D], FP8, kind="ExternalInput")
    Wo = nc.dram_tensor("Wo", [D, D], BF16, kind="ExternalInput")
    W1 = nc.dram_tensor("W1", [D, FF], BF16, kind="ExternalInput")
    W2 = nc.dram_tensor("W2", [FF, D], FP8, kind="ExternalInput")
    # biases arrive pre-transposed feature-major from the host: [P, n_ft]
    bq = nc.dram_tensor("bq", [P, FT], F32, kind="ExternalInput")
    bo = nc.dram_tensor("bo", [P, FT], F32, kind="ExternalInput")
    bf1 = nc.dram_tensor("bf1", [P, FFT], F32, kind="ExternalInput")
    bf2 = nc.dram_tensor("bf2", [P, FT], F32, kind="ExternalInput")
    selT_d = nc.dram_tensor("selT", [H, FT * P], BF16, kind="ExternalInput")
    out = nc.dram_tensor("out", [R, D], F32, kind="ExternalOutput")
    if debug_taps:
        dbg_sq = nc.dram_tensor("dbg_sq", [2, P, 512], F32, kind="ExternalOutput")
        dbg_ex = nc.dram_tensor("dbg_ex", [4, P, 512], BF16, kind="ExternalOutput")
        dbg_den = nc.dram_tensor("dbg_den", [H, R], F32, kind="ExternalOutput")
        dbg_hT = nc.dram_tensor("dbg_hT", [P, FT, R], BF16, kind="ExternalOutput")
        dbg_qT = nc.dram_tensor("dbg_qT", [P, FT, R], BF16, kind="ExternalOutput")
        dbg_kT = nc.dram_tensor("dbg_kT", [P, FT, RB], BF16, kind="ExternalOutput")

    inv_sqrt_e = 1.0 / float(np.sqrt(E))

    with tile.TileContext(nc) as tc:
        # --- pools with non-LIFO lifetimes: manual enter/exit (per side) ---
        def open_pool(name, bufs, space="SBUF", side="left"):
            cm = tc.tile_pool(name=name, bufs=bufs, space=space, side=side)
            return cm, cm.__enter__()

        def close_pool(cm):
            cm.__exit__(None, None, None)

        const_cm, const = open_pool("const", 1)

        # first x tiles before every other DMA: each dma_start costs ~600ns
        # of descriptor generation on the Sync queue; ~30 weight/bias loads
        # ahead of these would delay the first LN (and all PE work) by ~20us
        xpre = const.tile([P, 2, D], F32, tag="xpre")
        for t in (0, 1):
            nc.sync.dma_start(xpre[:, t, :], x_batch[t * P:(t + 1) * P, :])

        ident_bf = const.tile([P, P], BF16, tag="ident_bf")
        make_identity(nc, ident_bf)
        ident_f32 = const.tile([P, P], F32, tag="ident_f32")
        make_identity(nc, ident_f32)
        eps_t = const.tile([P, 1], F32, tag="eps")
        nc.vector.memset(eps_t[:], EPS)

        # head-select matrix for the denominator broadcast (host-built):
        # selT[h, f, j] = 1 iff h == 2f + (j >= E)
        selT = const.tile([H, FT, P], BF16, tag="selT")
        nc.sync.dma_start(selT[:],
                          selT_d.ap().rearrange("h (f p) -> h f p", p=P))



        # feature-major bias tiles [P, n_ft] (pre-transposed on host)
        def fmaj_bias(pool, name, src, n_ft):
            t = pool.tile([P, n_ft], F32, tag=name, name=name)
            nc.sync.dma_start(t[:], src.ap())
            return t

        bq8_t = fmaj_bias(const, "bq8", bq, FT)     # has 1/sqrt(E) folded in
        bo_t = fmaj_bias(const, "bo", bo, FT)
        bf1_t = fmaj_bias(const, "bf1", bf1, FFT)
        bf2_t = fmaj_bias(const, "bf2", bf2, FT)

        # layernorm (normalize only — gains/shifts are folded into the
        # weights/biases host-side) of one row-major [P, D] fp32 tile ->
        # bf16, transposed into dstT[:, f, r*P:(r+1)*P].
        def ln_tile(xb, dstT, r, scr, stat, tps):
            nch = max(1, D // 512)
            csz = D // nch
            st6 = stat.tile([P, nch, 6], F32, tag="st6", name="st6")
            for ci in range(nch):
                nc.vector.bn_stats(st6[:, ci, :], xb[:, ci * csz:(ci + 1) * csz])
            mv = stat.tile([P, 2], F32, tag="mv", name="mv")
            nc.vector.bn_aggr(mv[:], st6[:])
            sd = stat.tile([P, 1], F32, tag="sd", name="sd")
            nc.scalar.activation(sd[:], mv[:, 1:2], ACT.Sqrt, bias=eps_t[:])
            rstd = stat.tile([P, 1], F32, tag="rstd", name="rstd")
            nc.vector.reciprocal(rstd[:], sd[:])
            xn = scr.tile([P, D], BF16, tag="ln_xn", name="ln_xn")
            nc.vector.tensor_scalar(xn[:], xb[:], mv[:, 0:1], rstd[:],
                                    op0=OP.subtract, op1=OP.mult)
            for fb in range(0, FT, 4):
                nf = min(4, FT - fb)
                tp = tps.tile([P, nf * P], BF16, tag="tp_bf", name="tp_bf")
                for j in range(nf):
                    nc.tensor.transpose(tp[:, j * P:(j + 1) * P],
                                        xn[:, (fb + j) * P:(fb + j + 1) * P],
                                        ident_bf[:])
                nc.scalar.activation(
                    dstT[:, fb:fb + nf, r * P:(r + 1) * P],
                    tp.rearrange("p (f c) -> p f c", c=P), ACT.Copy)

        # stream a weight chunk (weights arrive pre-folded bf16 from host)
        def wchunk(wpool, dram, k, c0, csz, tag):
            wb = wpool.tile([P, csz], BF16, tag=tag + "_bf", name=tag)
            nc.sync.dma_start(wb[:], dram[k * P:(k + 1) * P, c0:c0 + csz])
            return wb

        # ============ Phase 1+2: LN1, V (interleaved), K, Q ============
        # own rows come FIRST in x_batch (host reorders), so xnT_b[:, :, :R]
        # doubles as the Q-projection input — no separate own-row LN.
        xnTb_cm, xnTb_pool = open_pool("xnTb", 1)
        xnT_b = xnTb_pool.tile([P, FT, RB], FP8, tag="xnT_b")
        att_cm, att_pool = open_pool("att", 1, side="right")
        kT = att_pool.tile([P, FT, RB], BF16, tag="kT")
        v_aug = att_pool.tile([P, RBT, H * (E + 1)], BF16, tag="v_aug")
        qT = att_pool.tile([P, FT, R], BF16, tag="qT")
        wv_cm, wv_pool = open_pool("wv", 1, side="right")
        Wv_bf = wv_pool.tile([P, FT, D], FP8, tag="Wv_bf")
        Wq_bf = wv_pool.tile([P, FT, D], FP8, tag="Wq_bf")
        Wk_bf = wv_pool.tile([P, FT, D], FP8, tag="Wk_bf")

        with tc.tile_pool(name="ln_x", bufs=4) as xpool, \
             tc.tile_pool(name="ln_scr", bufs=4) as scr, \
             tc.tile_pool(name="ln_stat", bufs=8) as stat, \
             tc.tile_pool(name="tps1", bufs=3, space="PSUM") as tps, \
             tc.tile_pool(name="mm2", bufs=5, space="PSUM") as mm:

            vgroups = _ngroups(D)

            # Wv upfront (V matmuls run inside the LN1 loop); Wq/Wk are
            # emitted after the LN loop to keep startup descriptor-gen short
            for k in range(FT):
                nc.sync.dma_start(Wv_bf[:, k, :], Wv[k * P:(k + 1) * P, :])

            def ln_v_tile(t):
                if t < 2:
                    xb = xpre[:, t, :]
                else:
                    xb = xpool.tile([P, D], F32, tag="ln_x", name="ln_x")
                    nc.sync.dma_start(xb[:], x_batch[t * P:(t + 1) * P, :])
                ln_tile(xb, xnT_b, t, scr, stat, tps)
                # V for this row tile (row-major, per-head ones column);
                # fp8 DoubleRow pairs two 128-deep k-tiles per instruction
                pss = [mm.tile([P, nsz], F32, name="mm2", tag="mm2")
                       for (_, nsz) in vgroups]
                for kk in range(FT // 2):
                    for ni, (n0, nsz) in enumerate(vgroups):
                        nc.tensor.matmul(pss[ni][:],
                                         xnT_b[:, 2 * kk:2 * kk + 2,
                                               t * P:(t + 1) * P],
                                         Wv_bf[:, 2 * kk:2 * kk + 2,
                                               n0:n0 + nsz],
                                         start=(kk == 0),
                                         stop=(kk == FT // 2 - 1),
                                         perf_mode=DR)
                va = v_aug[:, t, :].rearrange("p (h e) -> p h e", e=E + 1)
                for ni, (n0, nsz) in enumerate(vgroups):
                    hs = n0 // E
                    nh = nsz // E
                    if ni == 0:
                        nc.scalar.activation(
                            va[:, hs:hs + nh, 0:E],
                            pss[ni].rearrange("p (h e) -> p h e", e=E),
                            ACT.Copy, scale=1.0 / WSCALE)
                    else:
                        nc.vector.tensor_scalar_mul(
                            va[:, hs:hs + nh, 0:E],
                            pss[ni].rearrange("p (h e) -> p h e", e=E),
                            1.0 / WSCALE)
                nc.vector.memset(va[:, :, E:E + 1], 1.0)

            for t in range(RBT):
                ln_v_tile(t)

            for k in range(FT):
                nc.sync.dma_start(Wk_bf[:, k, :], Wk[k * P:(k + 1) * P, :])
            for k in range(FT):
                nc.sync.dma_start(Wq_bf[:, k, :], Wq[k * P:(k + 1) * P, :])

            # kT (feature-major)
            kgroups = _ngroups(RB)
            for f in range(FT):
                pss = [mm.tile([P, nsz], F32, name="mm2", tag="mm2")
                       for (_, nsz) in kgroups]
                for kk in range(FT // 2):
                    for ni, (n0, nsz) in enumerate(kgroups):
                        nc.tensor.matmul(pss[ni][:],
                                         Wk_bf[:, 2 * kk:2 * kk + 2,
                                               f * P:(f + 1) * P],
                                         xnT_b[:, 2 * kk:2 * kk + 2,
                                               n0:n0 + nsz],
                                         start=(kk == 0),
                                         stop=(kk == FT // 2 - 1),
                                         perf_mode=DR)
                for ni, (n0, nsz) in enumerate(kgroups):
                    nc.vector.tensor_scalar_mul(kT[:, f, n0:n0 + nsz],
                                                pss[ni][:], 1.0 / WSCALE)

            # qT: compact feature-major q with 1/sqrt(E) and bq folded in.
            # Head h lives at partitions (h%2)*E .. +E of slice f=h//2 —
            # exactly the projection output layout, no interleave needed.
            qgroups = _ngroups(R)
            for f in range(FT):
                pss = [mm.tile([P, nsz], F32, name="mm2", tag="mm2")
                       for (_, nsz) in qgroups]
                for kk in range(FT // 2):
                    for ni, (n0, nsz) in enumerate(qgroups):
                        nc.tensor.matmul(pss[ni][:],
                                         Wq_bf[:, 2 * kk:2 * kk + 2,
                                               f * P:(f + 1) * P],
                                         xnT_b[:, 2 * kk:2 * kk + 2,
                                               n0:n0 + nsz],
                                         start=(kk == 0),
                                         stop=(kk == FT // 2 - 1),
                                         perf_mode=DR)
                for ni, (n0, nsz) in enumerate(qgroups):
                    nc.scalar.activation(qT[:, f, n0:n0 + nsz], pss[ni][:],
                                         ACT.Identity, bias=bq8_t[:, f:f + 1],
                                         scale=inv_sqrt_e / WSCALE)
        close_pool(wv_cm)
        close_pool(xnTb_cm)

        # ============ Phase 3: attention ============
        hT_cm, hT_pool = open_pool("hT", 1)
        hT = hT_pool.tile([P, FT, R], BF16, tag="hT")
        den_cm, den_pool = open_pool("den", 1, side="right")
        den = den_pool.tile([H, R], F32, tag="den")
        den_hbm = nc.dram_tensor("den_hbm", [H, R], F32)
        wo_cm, wo_pool = open_pool("wo", 1)
        Wo_bf = wo_pool.tile([P, FT, D], BF16, tag="Wo_bf")
        for k in range(FT):
            nc.sync.dma_start(Wo_bf[:, k, :], Wo[k * P:(k + 1) * P, :])

        if debug_taps:
            nc.sync.dma_start(dbg_qT[:], qT[:])
            nc.sync.dma_start(dbg_kT[:], kT[:])

        qgroups = _ngroups(R)
        with tc.tile_pool(name="spsum", bufs=4, space="PSUM") as spool, \
             tc.tile_pool(name="opsum", bufs=4, space="PSUM") as opool, \
             tc.tile_pool(name="dstage", bufs=8) as dspool, \
             tc.tile_pool(name="expool", bufs=8) as expool:
            for hp in range(0, H, 2):
                heads = [hp, hp + 1]
                for qi, (q0, qsz) in enumerate(qgroups):
                    o_pss = {h: opool.tile([E + 1, qsz], F32, name="o",
                                           tag="o") for h in heads}
                    av_prev = None
                    for tp_i in range(RBT // 2):
                        # two k-tiles per step: scores for both t's first,
                        # then the previous step's four AV matmuls — fewer
                        # score<->AV shape transitions on the PE pipeline
                        ts2 = (2 * tp_i, 2 * tp_i + 1)
                        sq = {}
                        for t in ts2:
                            for h in heads:
                                ph = (h % HPT) * E
                                fh = h // HPT
                                s = spool.tile([P, 512], F32, name="sq",
                                               tag="sq")
                                nc.tensor.matmul(
                                    s[:, :qsz],
                                    kT[ph:ph + E, fh, t * P:(t + 1) * P],
                                    qT[ph:ph + E, fh, q0:q0 + qsz],
                                    start=True, stop=True)
                                sq[(h, t)] = s
                        # exp: ScalarE exact on even head, DVE bit-trick on
                        # odd head (rows stay on one engine for num/den
                        # error cancellation)
                        ex = {}
                        for t in ts2:
                            for ih, h in enumerate(heads):
                                e_t = expool.tile([P, 512], BF16, name="ex",
                                                  tag="ex")
                                if dve_exp and ih == 1:
                                    di = e_t[:, :qsz].bitcast(I16)
                                    nc.vector.tensor_scalar(
                                        di, sq[(h, t)][:, :qsz], FE_A, FE_B,
                                        op0=OP.mult, op1=OP.add)
                                else:
                                    nc.scalar.activation(e_t[:, :qsz],
                                                         sq[(h, t)][:, :qsz],
                                                         ACT.Exp)
                                ex[(h, t)] = e_t
                        if debug_taps and hp == 0 and qi == 0 and tp_i == 0:
                            for ih, h in enumerate(heads):
                                st = dspool.tile([P, 512], F32, tag="dbg_s")
                                nc.vector.tensor_scalar_mul(st[:],
                                                            sq[(h, 0)][:, :],
                                                            1.0)
                                nc.sync.dma_start(dbg_sq[ih, :, :], st[:])
                                nc.sync.dma_start(dbg_ex[ih, :, :],
                                                  ex[(h, 0)][:])
                        if av_prev is not None:
                            tsp, ex_ = av_prev
                            for h in heads:
                                for t_ in tsp:
                                    nc.tensor.matmul(
                                        o_pss[h][:, :qsz],
                                        v_aug[:, t_,
                                              h * (E + 1):(h + 1) * (E + 1)],
                                        ex_[(h, t_)][:, :qsz],
                                        start=(t_ == 0), stop=False)
                        av_prev = (ts2, ex)
                    tsp, ex_ = av_prev
                    for h in heads:
                        for t_ in tsp:
                            nc.tensor.matmul(
                                o_pss[h][:, :qsz],
                                v_aug[:, t_, h * (E + 1):(h + 1) * (E + 1)],
                                ex_[(h, t_)][:, :qsz],
                                start=(t_ == 0), stop=(t_ == RBT - 1))
                    # evict unnormalized h + denominator row; engines are
                    # split per head so the two evictions run in parallel
                    for ih, h in enumerate(heads):
                        p_h = (h % HPT) * E
                        f_h = h // HPT
                        o_ps = o_pss[h]
                        hslc = hT[p_h:p_h + E, f_h, q0:q0 + qsz]
                        ds = dspool.tile([1, 512], F32, name="ds", tag="ds")
                        if ih == 0:
                            nc.scalar.activation(hslc, o_ps[0:E, :], ACT.Copy)
                            nc.vector.tensor_scalar_mul(ds[:, :qsz],
                                                        o_ps[E:E + 1, :], 1.0)
                        else:
                            nc.vector.tensor_scalar_mul(hslc, o_ps[0:E, :],
                                                        1.0)
                            nc.scalar.activation(ds[:, :qsz],
                                                 o_ps[E:E + 1, :], ACT.Copy)
                        nc.sync.dma_start(
                            den_hbm.ap()[h:h + 1, q0:q0 + qsz], ds[:, :qsz])

        # deferred softmax normalization: one reciprocal, select-matmul
        # broadcast, one multiply per (f, qgroup).  den rows were staged
        # through DRAM (engines cannot scatter to unaligned partition bases;
        # the DMA round-trip redistributes them onto partitions 0..H-1).
        if debug_taps:
            nc.sync.dma_start(dbg_hT[:], hT[:])
        nc.sync.dma_start(den[:], den_hbm.ap())
        den_r = den_pool.tile([H, R], BF16, tag="den_r")
        with nc.allow_low_precision("softmax 1/den in bf16"):
            nc.vector.reciprocal(den_r[:], den[:])
        with tc.tile_pool(name="rbps", bufs=4, space="PSUM") as rbpool:
            for f in range(FT):
                for qi, (q0, qsz) in enumerate(qgroups):
                    rb = rbpool.tile([P, qsz], F32, name="rb", tag="rb")
                    nc.tensor.matmul(rb[:], selT[:, f, :],
                                     den_r[:, q0:q0 + qsz],
                                     start=True, stop=True)
                    nc.vector.tensor_tensor(hT[:, f, q0:q0 + qsz],
                                            hT[:, f, q0:q0 + qsz],
                                            rb[:], op=OP.mult)
        close_pool(den_cm)
        close_pool(att_cm)

        # ============ Phase 4+5: Wo projection + residual, LN2 (pipelined) ==
        y1_cm, y1_pool = open_pool("y1", 1, side="right")
        y1 = y1_pool.tile([P, R // P, D], F32, tag="y1")
        xn2_cm, xn2_pool = open_pool("xn2T", 1, side="right")
        xn2T = xn2_pool.tile([P, FT, R], BF16, tag="xn2T")

        with tc.tile_pool(name="ln_scr2", bufs=4) as scr2, \
             tc.tile_pool(name="ln_stat2", bufs=8) as stat2, \
             tc.tile_pool(name="ev4", bufs=4) as ev4, \
             tc.tile_pool(name="xres", bufs=4) as xres, \
             tc.tile_pool(name="mm4", bufs=4, space="PSUM") as mm4, \
             tc.tile_pool(name="tp4", bufs=2, space="PSUM") as tp4:
            for ni, (n0, nsz) in enumerate(_ngroups(R)):
                nj = nsz // P
                for f in range(FT):
                    ps = mm4.tile([P, nsz], F32, name="mm4", tag="mm4")
                    for k in range(FT):
                        nc.tensor.matmul(ps[:], Wo_bf[:, k, f * P:(f + 1) * P],
                                         hT[:, k, n0:n0 + nsz],
                                         start=(k == 0), stop=(k == FT - 1))
                    pe = ev4.tile([P, nsz], F32, name="pe", tag="pe")
                    nc.scalar.activation(pe[:], ps[:], ACT.Identity,
                                         bias=bo_t[:, f:f + 1])
                    tp = tp4.tile([P, nsz], F32, name="tp4", tag="tp4")
                    for j in range(nj):
                        nc.tensor.transpose(tp[:, j * P:(j + 1) * P],
                                            pe[:, j * P:(j + 1) * P],
                                            ident_f32[:])
                    xo = xres.tile([P, nj, P], F32, name="xo", tag="xo")
                    nc.sync.dma_start(
                        xo[:], x_own[n0:n0 + nsz, f * P:(f + 1) * P]
                        .rearrange("(j p) c -> p j c", p=P))
                    nc.vector.tensor_tensor(
                        y1[:, n0 // P:n0 // P + nj, f * P:(f + 1) * P],
                        tp.rearrange("p (j c) -> p j c", c=P), xo[:], op=OP.add)
                # LN2 for the rows of this group (overlaps next group's PE)
                for r in range(n0 // P, (n0 + nsz) // P):
                    ln_tile(y1[:, r, :], xn2T, r, scr2, stat2, tp4)
        close_pool(wo_cm)
        close_pool(hT_cm)

        # ============ Phase 6: FFN up + gelu ============
        ff1_cm, ff1_pool = open_pool("ff1T", 1)
        ff1T = ff1_pool.tile([P, FFT // 2, 2, R], FP8, tag="ff1T")
        with tc.tile_pool(name="w_1", bufs=8) as wpool6, \
             tc.tile_pool(name="mm6", bufs=8, space="PSUM") as mm6:
            for fb in range(0, FFT, 2):
                groups = _ngroups(R)
                pss = {}
                for mi in range(2):
                    for ni, (n0, nsz) in enumerate(groups):
                        pss[(mi, ni)] = mm6.tile([P, nsz], F32, name="mm6",
                                                 tag="mm6")
                for k in range(FT):
                    wb = wchunk(wpool6, W1, k, fb * P, 2 * P, "w1")
                    for mi in range(2):
                        for ni, (n0, nsz) in enumerate(groups):
                            nc.tensor.matmul(pss[(mi, ni)][:],
                                             wb[:, mi * P:(mi + 1) * P],
                                             xn2T[:, k, n0:n0 + nsz],
                                             start=(k == 0), stop=(k == FT - 1))
                for mi in range(2):
                    f = fb + mi
                    for ni, (n0, nsz) in enumerate(groups):
                        dst = ff1T[:, f // 2, f % 2, n0:n0 + nsz]
                        if not sim_safe_gelu:
                            nc.scalar.activation(dst, pss[(mi, ni)][:],
                                                 ACT.Gelu,
                                                 bias=bf1_t[:, f:f + 1])
                        else:
                            _gelu_tanh(nc, tc, dst, pss[(mi, ni)][:],
                                       bf1_t[:, f:f + 1], P, nsz)

        # ============ Phase 7: FFN down + residual -> out ============
        with tc.tile_pool(name="w_2", bufs=8) as wpool7, \
             tc.tile_pool(name="ev7", bufs=4) as ev7, \
             tc.tile_pool(name="ob7", bufs=4) as ob7, \
             tc.tile_pool(name="mm7", bufs=5, space="PSUM") as mm7, \
             tc.tile_pool(name="tp7", bufs=3, space="PSUM") as tp7:
            for fb in range(0, FT, 2):
                groups = _ngroups(R)
                pss = {}
                for mi in range(2):
                    for ni, (n0, nsz) in enumerate(groups):
                        pss[(mi, ni)] = mm7.tile([P, nsz], F32, name="mm7",
                                                 tag="mm7")
                for kk in range(FFT // 2):
                    wb = wpool7.tile([P, 2, 2 * P], FP8, tag="w2_bf",
                                     name="w2")
                    nc.sync.dma_start(
                        wb[:], W2[2 * kk * P:(2 * kk + 2) * P,
                                  fb * P:(fb + 2) * P]
                        .rearrange("(j p) c -> p j c", p=P))
                    for mi in range(2):
                        for ni, (n0, nsz) in enumerate(groups):
                            nc.tensor.matmul(pss[(mi, ni)][:],
                                             wb[:, :, mi * P:(mi + 1) * P],
                                             ff1T[:, kk, :, n0:n0 + nsz],
                                             start=(kk == 0),
                                             stop=(kk == FFT // 2 - 1),
                                             perf_mode=DR)
                for mi in range(2):
                    f = fb + mi
                    for ni, (n0, nsz) in enumerate(groups):
                        nj = nsz // P
                        pe = ev7.tile([P, nsz], F32, name="pe7", tag="pe7")
                        nc.vector.tensor_scalar(pe[:], pss[(mi, ni)][:],
                                                1.0 / WSCALE,
                                                bf2_t[:, f:f + 1],
                                                op0=OP.mult, op1=OP.add)
                        tp = tp7.tile([P, nsz], F32, name="tp7", tag="tp7")
                        for j in range(nj):
                            nc.tensor.transpose(tp[:, j * P:(j + 1) * P],
                                                pe[:, j * P:(j + 1) * P],
                                                ident_f32[:])
                        ob = ob7.tile([P, nj, P], F32, name="ob", tag="ob")
                        nc.vector.tensor_tensor(
                            ob[:], tp.rearrange("p (j c) -> p j c", c=P),
                            y1[:, n0 // P:n0 // P + nj, f * P:(f + 1) * P],
                            op=OP.add)
                        nc.sync.dma_start(
                            out[n0:n0 + nsz, f * P:(f + 1) * P]
                            .rearrange("(j p) c -> p j c", p=P), ob[:])
        close_pool(ff1_cm)
        close_pool(xn2_cm)
        close_pool(y1_cm)
        close_pool(const_cm)

    nc.compile()
    return nc


def _gelu_tanh(nc, tc, out_ap, ps, bias_col, p, nsz):
    """CoreSim-safe tanh gelu: 0.5*x*(1+tanh(0.79788456*(x+0.044715*x^3)))."""
    with tc.tile_pool(name="gelu_scr", bufs=2) as gs:
        x = gs.tile([p, nsz], F32, tag="g_x", name="g_x")
        nc.vector.tensor_scalar(x[:], ps[:], bias_col, None, op0=OP.add)
        x3 = gs.tile([p, nsz], F32, tag="g_x3", name="g_x3")
        nc.vector.tensor_tensor(x3[:], x[:], x[:], op=OP.mult)
        nc.vector.tensor_tensor(x3[:], x3[:], x[:], op=OP.mult)
        nc.vector.tensor_scalar(x3[:], x3[:], 0.044715, None, op0=OP.mult)
        nc.vector.tensor_tensor(x3[:], x3[:], x[:], op=OP.add)
        th = gs.tile([p, nsz], F32, tag="g_th", name="g_th")
        nc.scalar.activation(th[:], x3[:], ACT.Tanh, scale=0.7978845608028654)
        nc.vector.tensor_scalar(th[:], th[:], 1.0, 0.5, op0=OP.add, op1=OP.mult)
        nc.vector.tensor_tensor(out_ap, x[:], th[:], op=OP.mult)


# ---------------- host-side driver ----------------

_COMPILED = {}

_B, _S, _D, _H, _E, _FF = 4, 2048, 1024, 16, 64, 4096
_NCORES = 8
_R = (_B * _S) // _NCORES          # 1024 own rows per core
_CPB = _NCORES // _B               # cores per batch


def _get_nc():
    key = "full"
    if key not in _COMPILED:
        _COMPILED[key] = build_nc(R=_R, RB=_S, D=_D, H=_H, E=_E, FF=_FF,
                                  n_cores=_NCORES)
    return _COMPILED[key]


def _fmaj(b):
    """[D] bias -> feature-major [P, D//P]: elem [p, f] = b[f*128+p]."""
    return np.ascontiguousarray(b.reshape(-1, P).T.astype(np.float32))


def fold_params(inputs):
    """Weight-only reparametrization: fold LN gains/shifts into the adjacent
    matmul weights/biases and pre-cast weights to bf16.
      LN(x;g,b) @ W + c  ==  z @ (g*W) + (b@W + c),  z = (x-mu)*rstd
    bk is dropped entirely (softmax is invariant to per-query shifts) and
    bv is folded through the attention average into bo:
      y1 = x + A(V)@Wo + bo  with V = xn@Wv' + bv'  ->  bo' = bo + bv'@Wo."""
    import ml_dtypes
    f = lambda n: np.asarray(inputs[n], dtype=np.float32)
    g1, b1, g2, b2 = f("g1"), f("b1"), f("g2"), f("b2")
    Wq, Wk, Wv, Wo = f("Wq"), f("Wk"), f("Wv"), f("Wo")
    W1, W2 = f("W1"), f("W2")
    bvp = f("bv") + b1 @ Wv
    bf16 = ml_dtypes.bfloat16
    inv_sqrt_e = 1.0 / float(np.sqrt(_E))
    fp8 = ml_dtypes.float8_e4m3
    H, E = _H, _E
    sel = np.zeros((H, H * E // P, P), np.float32)
    for ff in range(H * E // P):
        sel[2 * ff, ff, 0:E] = 1.0
        sel[2 * ff + 1, ff, E:P] = 1.0

    def wq8(w):
        return np.ascontiguousarray(
            np.clip(w * WSCALE, -240.0, 240.0).astype(fp8))

    out = {
        "selT": np.ascontiguousarray(sel.reshape(H, -1).astype(bf16)),
        "Wq": wq8(g1[:, None] * Wq),
        "Wk": wq8(g1[:, None] * Wk),
        "Wv": wq8(g1[:, None] * Wv),
        "Wo": np.ascontiguousarray(Wo.astype(bf16)),
        "W1": np.ascontiguousarray((g2[:, None] * W1).astype(bf16)),
        "W2": wq8(W2),
        "bq": _fmaj((f("bq") + b1 @ Wq) * inv_sqrt_e),
        "bo": _fmaj(f("bo") + bvp @ Wo),
        "bf1": _fmaj(f("bf1") + b2 @ W1),
        "bf2": _fmaj(f("bf2")),
    }
    return out


def make_in_maps(inputs):
    """Build the 8 per-core input maps.  x_batch is the core's whole batch
    with its OWN rows rotated to the front (attention is permutation-
    invariant over keys, and the kernel assumes own rows lead)."""
    x = np.ascontiguousarray(np.asarray(inputs["x"], dtype=np.float32))
    xf = x.reshape(_NCORES, _R, _D)
    xb = x.reshape(_B, _S, _D)
    shared = fold_params(inputs)
    in_maps = []
    for c in range(_NCORES):
        m = dict(shared)
        m["x_own"] = xf[c]
        b = c // _CPB
        half = c % _CPB
        other = xb[b][(1 - half) * _R:(2 - half) * _R]
        m["x_batch"] = np.ascontiguousarray(
            np.concatenate([xf[c], other], axis=0))
        in_maps.append(m)
    return in_maps


def kernel(**inputs):
    nc = _get_nc()
    in_maps = make_in_maps(inputs)
    res = run_bass_kernel_spmd(nc, in_maps, core_ids=list(range(_NCORES)))
    out = np.concatenate([res.results[c]["out"] for c in range(_NCORES)], axis=0)
    return out.reshape(_B, _S, _D).astype(np.float32)
